# revision 9
# baseline (speedup 1.0000x reference)
"""Trainium2 Bass kernel for nn_Aggregator (BN1d + Swish + Linear + relevance-weighted head sum).

out[b, :] = sum_h w[b,h] * (silu(x[b,h,:] * inv + shift) @ W.T + bias)
          = (sum_h w[b,h] * silu(x[b,h,:] * inv + shift)) @ W.T + (sum_h w[b,h]) * bias

Data parallel over 8 NeuronCores: batch dim B split 8 ways, all params replicated.

v2 design — bf16 streaming + feature-transposed layout:
  - x is cast to bf16 host-side (tolerance 2e-2 >> bf16 error ~5e-3): halves
    the mandatory HBM read from 16.8 MB to 8.4 MB per core.
  - Features on PARTITIONS (d = c*128 + p), rows on the free dim. Per-core
    free layout per superblock (sb = 256 b-values): [c(4), nn(2), h(8), b(128)].
  - BN affine = one dual-scalar DVE/GPSIMD tensor_scalar per c-chunk
    (scale=inv_p, then add shift_p), 2x bf16 rate on DVE.
  - Swish = ONE big ACT instruction per superblock (N=8192/partition) —
    amortizes the 352-cycle ACT instruction overhead. ACT is the bottleneck
    engine at ~28.5 us/rep.
  - Weighted head-sum: DVE mult by w (broadcast across partitions) + 3-level
    contiguous tree add over h. No PE staircase, no transposes.
  - Linear: 16 small matmuls/sb (stationary W chunk [128d,128do], moving
    g [128d,128b]) accumulating over c in PSUM. PE ~5 us total.
  - bias: out = psum + b_do * sumw_b via one scalar_tensor_tensor per half.
  - Output stored bf16, transposed [do, b]; host unscrambles + upcasts.
"""

import os
from contextlib import ExitStack

import numpy as np

import concourse.bacc as bacc
import concourse.mybir as mybir
import concourse.tile as tile
from concourse.bass_utils import run_bass_kernel_spmd
from concourse.mybir import AluOpType

N_CORES = 8
B, H, D, DO = 8192, 8, 512, 256
B_LOC = B // N_CORES            # 1024 b-values per core
NSB = 4                         # superblocks of 256 b-values
SB_B = B_LOC // NSB             # 256 b per superblock
NN = 2                          # blocks of 128 b per superblock
NC_CHUNK = 4                    # feature chunks of 128
FREE = NC_CHUNK * NN * H * 128  # 8192 elems per partition per superblock
EPS = 1e-5
FP = mybir.dt.float32
BF = mybir.dt.bfloat16

# Which BN c-chunks run on GPSIMD (rest on DVE). Balancing knob.
GP_BN_CS = (2, 3)


ALL_STAGES = frozenset({"bn", "silu", "wmul", "tree", "mm", "tail"})


def build_kernel(repeat: int = 1, stages: frozenset = ALL_STAGES):
    """repeat>1 re-runs the whole superblock loop (same I/O) for slope timing.

    stages: ablation knob for timing attribution; dropping stages yields a
    garbage-output (but schedulable) kernel.
    """
    nc = bacc.Bacc(
        "TRN2",
        target_bir_lowering=False,
        debug=False,
        num_devices=N_CORES,
    )

    x_d = nc.dram_tensor("x", (NSB, 128, FREE), BF, kind="ExternalInput")
    wb_d = nc.dram_tensor("wb", (128, NSB * NN * H * 128), BF, kind="ExternalInput")
    sumw_d = nc.dram_tensor("sumw", (128, NSB * NN * 128), FP, kind="ExternalInput")
    invT_d = nc.dram_tensor("invT", (128, NC_CHUNK), FP, kind="ExternalInput")
    shiftT_d = nc.dram_tensor("shiftT", (128, NC_CHUNK), FP, kind="ExternalInput")
    wt_d = nc.dram_tensor("wt", (128, 2 * NC_CHUNK * 128), BF, kind="ExternalInput")
    bvec_d = nc.dram_tensor("bvec", (128, 2), FP, kind="ExternalInput")
    out_d = nc.dram_tensor("out", (NSB, NN, 128, DO), BF, kind="ExternalOutput")

    with tile.TileContext(nc) as tc, ExitStack() as ctx:
        const = ctx.enter_context(tc.tile_pool(name="const", bufs=1))
        xpool = ctx.enter_context(tc.tile_pool(name="xin", bufs=3))

        # first superblock load precedes const loads in the SP FIFO except the
        # tiny BN params the first compute depends on
        invT = const.tile([128, NC_CHUNK], FP)
        nc.sync.dma_start(invT[:], invT_d.ap())
        shiftT = const.tile([128, NC_CHUNK], FP)
        nc.sync.dma_start(shiftT[:], shiftT_d.ap())
        xt0 = xpool.tile([128, FREE], BF, tag="xt")
        nc.sync.dma_start(xt0[:], x_d.ap()[0])
        wbc = const.tile([128, NSB * NN * H * 128], BF)
        nc.sync.dma_start(wbc[:], wb_d.ap())
        wt = const.tile([128, 2 * NC_CHUNK * 128], BF)
        nc.sync.dma_start(wt[:], wt_d.ap())
        sumw = const.tile([128, NSB * NN * 128], FP)
        nc.sync.dma_start(sumw[:], sumw_d.ap())
        bvec = const.tile([128, 2], FP)
        nc.sync.dma_start(bvec[:], bvec_d.ap())

        tpool = ctx.enter_context(tc.tile_pool(name="bn", bufs=2))
        hspool = ctx.enter_context(tc.tile_pool(name="hs", bufs=2))
        mpool = ctx.enter_context(tc.tile_pool(name="m", bufs=2))
        t4pool = ctx.enter_context(tc.tile_pool(name="t4", bufs=2))
        t2pool = ctx.enter_context(tc.tile_pool(name="t2", bufs=2))
        gpool = ctx.enter_context(tc.tile_pool(name="g", bufs=2))
        opool = ctx.enter_context(tc.tile_pool(name="o", bufs=4))
        pspool = ctx.enter_context(tc.tile_pool(name="ps", bufs=2, space="PSUM"))

        CH = FREE // NC_CHUNK  # 2048 elems per c-chunk per partition

        for rep in range(repeat):
          for sb in range(NSB):
            if sb == 0 and rep == 0:
                xt = xt0
            else:
                xt = xpool.tile([128, FREE], BF, tag="xt")
                nc.sync.dma_start(xt[:], x_d.ap()[sb])

            # BN affine per c-chunk: t = x * inv_p + shift_p
            t = tpool.tile([128, FREE], BF, tag="t")
            if "bn" in stages:
                for c in range(NC_CHUNK):
                    eng = nc.gpsimd if c in GP_BN_CS else nc.vector
                    eng.tensor_scalar(
                        t[:, c * CH : (c + 1) * CH],
                        xt[:, c * CH : (c + 1) * CH],
                        invT[:, c : c + 1],
                        shiftT[:, c : c + 1],
                        AluOpType.mult,
                        AluOpType.add,
                    )

            # Swish, one big ACT op
            hs = hspool.tile([128, FREE], BF, tag="hs")
            if "silu" in stages:
                src = t if "bn" in stages else xt
                nc.scalar.activation(
                    hs[:], src[:], mybir.ActivationFunctionType.Silu
                )

            # weighted by relevance: m = hs * w (w broadcast over partitions+c)
            m = mpool.tile([128, FREE], BF, tag="m")
            if "wmul" in stages:
                src = hs if "silu" in stages else xt
                wslice = (
                    wbc[:, sb * CH : (sb + 1) * CH]
                    .unsqueeze(1)
                    .broadcast_to([128, NC_CHUNK, CH])
                )
                nc.vector.tensor_tensor(
                    m[:].rearrange("p (c r) -> p c r", c=NC_CHUNK),
                    src[:].rearrange("p (c r) -> p c r", c=NC_CHUNK),
                    wslice,
                    AluOpType.mult,
                )

            # head-sum: 3-level tree over h (cn = 2c+nn merged dim of 8)
            if "tree" in stages:
                g = gpool.tile([128, 8 * 128], BF, tag="g")
                src = m if "wmul" in stages else xt
                mv = src[:].rearrange("p (cn h b) -> p cn h b", cn=8, h=8)
                t4 = t4pool.tile([128, 8 * 4 * 128], BF, tag="t4")
                t4v = t4[:].rearrange("p (cn h b) -> p cn h b", cn=8, h=4)
                nc.vector.tensor_tensor(
                    t4v, mv[:, :, 0:4, :], mv[:, :, 4:8, :], AluOpType.add
                )
                t2t = t2pool.tile([128, 8 * 2 * 128], BF, tag="t2")
                t2v = t2t[:].rearrange("p (cn h b) -> p cn h b", cn=8, h=2)
                nc.vector.tensor_tensor(
                    t2v, t4v[:, :, 0:2, :], t4v[:, :, 2:4, :], AluOpType.add
                )
                gv = g[:].rearrange("p (cn h b) -> p cn h b", cn=8, h=1)
                nc.vector.tensor_tensor(
                    gv, t2v[:, :, 0:1, :], t2v[:, :, 1:2, :], AluOpType.add
                )

            # Linear: accumulate over c into PSUM per (nn, half)
            ps = [
                pspool.tile([128, 128], FP, tag=f"ps{i}", name=f"ps{i}")
                for i in range(NN * 2)
            ]
            if "mm" in stages:
                for c in range(NC_CHUNK):
                    for half in range(2):
                        wsl = wt[:, (2 * c + half) * 128 : (2 * c + half + 1) * 128]
                        for n2 in range(NN):
                            cn = 2 * c + n2
                            rhs = (
                                g[:, cn * 128 : (cn + 1) * 128]
                                if "tree" in stages
                                else wt[:, cn * 128 : (cn + 1) * 128]
                            )
                            nc.tensor.matmul(
                                ps[n2 * 2 + half][:],
                                wsl,
                                rhs,
                                start=(c == 0),
                                stop=(c == NC_CHUNK - 1),
                            )

            # bias: out = psum + b_do * sumw_b ; store bf16 via ACT HWDGE queue
            if "tail" in stages:
                for n2 in range(NN):
                    o = opool.tile([128, DO], BF, tag="o")
                    for half in range(2):
                        if "mm" in stages:
                            nc.vector.scalar_tensor_tensor(
                                o[:, half * 128 : (half + 1) * 128],
                                sumw[:, (sb * NN + n2) * 128 : (sb * NN + n2 + 1) * 128],
                                bvec[:, half : half + 1],
                                ps[n2 * 2 + half][:],
                                AluOpType.mult,
                                AluOpType.add,
                            )
                        else:
                            nc.vector.tensor_scalar(
                                o[:, half * 128 : (half + 1) * 128],
                                wt[:, half * 128 : (half + 1) * 128],
                                bvec[:, half : half + 1],
                                None,
                                AluOpType.mult,
                            )
                    nc.scalar.dma_start(out_d.ap()[sb][n2], o[:])

    nc.compile()
    return nc


def make_host_inputs(x_np, w_np, gamma, beta, mean, var, W, b):
    """Build the per-core input maps (host-side layout prep only)."""
    import ml_dtypes

    BFH = ml_dtypes.bfloat16
    inv = (gamma / np.sqrt(var + EPS)).astype(np.float32)
    shift = (beta - mean * inv).astype(np.float32)
    invT = np.ascontiguousarray(inv.reshape(NC_CHUNK, 128).T)
    shiftT = np.ascontiguousarray(shift.reshape(NC_CHUNK, 128).T)
    # Wt[p, (c*2+half)*128 + q] = W[half*128+q, c*128+p]
    wt = np.ascontiguousarray(
        W.astype(np.float32)
        .reshape(2, 128, NC_CHUNK, 128)  # [half, q, c, p]
        .transpose(3, 2, 0, 1)           # [p, c, half, q]
        .reshape(128, 2 * NC_CHUNK * 128)
        .astype(BFH)
    )
    bvec = np.ascontiguousarray(b.astype(np.float32).reshape(2, 128).T)

    xb = x_np.astype(BFH)  # round once, globally
    in_maps = []
    for core in range(N_CORES):
        b0 = core * B_LOC
        # x_t[sb, p, c, nn, h, bb] = x[b0 + sb*256 + nn*128 + bb, h, c*128 + p]
        x_core = xb[b0 : b0 + B_LOC].reshape(NSB, NN, 128, H, NC_CHUNK, 128)
        x_t = np.ascontiguousarray(
            x_core.transpose(0, 5, 4, 1, 3, 2).reshape(NSB, 128, FREE)
        )
        w_core = w_np[b0 : b0 + B_LOC].astype(np.float32)
        # wb[p, sb, nn, h, bb]
        w_r = (
            w_core.reshape(NSB, NN, 128, H).transpose(0, 1, 3, 2).reshape(-1)
        )
        wb = np.ascontiguousarray(
            np.broadcast_to(w_r.astype(BFH), (128, NSB * NN * H * 128))
        )
        sumw_r = w_core.sum(axis=1).astype(np.float32)  # [1024] = [sb, nn, bb]
        sumw = np.ascontiguousarray(np.broadcast_to(sumw_r, (128, NSB * NN * 128)))
        in_maps.append(
            {
                "x": x_t,
                "wb": wb,
                "sumw": sumw,
                "invT": invT,
                "shiftT": shiftT,
                "wt": wt,
                "bvec": bvec,
            }
        )
    return in_maps


_NC_CACHE = None
LAST_RESULT = None


def make_runner(nc, in_maps):
    """Build a reusable jitted SPMD callable with device-resident inputs.

    Mirrors bass2jax.run_bass_via_pjrt's multi-core path, but without
    donation so the same device buffers can be executed repeatedly for
    steady-state timing.
    """
    import jax
    from concourse import bass2jax
    from jax.experimental.shard_map import shard_map
    from jax.sharding import Mesh, NamedSharding, PartitionSpec

    bass2jax.install_neuronx_cc_hook()
    partition_name = nc.partition_id_tensor.name if nc.partition_id_tensor else None
    in_names, out_names, out_avals, zero_outs = [], [], [], []
    for alloc in nc.m.functions[0].allocations:
        if not isinstance(alloc, mybir.MemoryLocationSet):
            continue
        name = alloc.memorylocations[0].name
        if alloc.kind == "ExternalInput":
            if name != partition_name:
                in_names.append(name)
        elif alloc.kind == "ExternalOutput":
            out_names.append(name)
            shape = tuple(alloc.tensor_shape)
            dtype = mybir.dt.np(alloc.dtype)
            out_avals.append(jax.core.ShapedArray(shape, dtype))
            zero_outs.append(np.zeros(shape, dtype))
    n_params = len(in_names)
    all_names = in_names + out_names
    if partition_name is not None:
        all_names = all_names + [partition_name]

    def _body(*args):
        operands = list(args)
        if partition_name is not None:
            operands.append(bass2jax.partition_id_tensor())
        outs = bass2jax._bass_exec_p.bind(
            *operands,
            out_avals=tuple(out_avals),
            in_names=tuple(all_names),
            out_names=tuple(out_names),
            lowering_input_output_aliases=(),
            sim_require_finite=True,
            sim_require_nnan=True,
            nc=nc,
        )
        return tuple(outs)

    n_cores = len(in_maps)
    devices = jax.devices()[:n_cores]
    mesh = Mesh(np.asarray(devices), ("core",))
    in_specs = (PartitionSpec("core"),) * (n_params + len(out_names))
    out_specs = (PartitionSpec("core"),) * len(out_names)
    fn = jax.jit(
        shard_map(_body, mesh=mesh, in_specs=in_specs, out_specs=out_specs,
                  check_rep=False),
        keep_unused=True,
    )
    sh = NamedSharding(mesh, PartitionSpec("core"))
    concat = [
        np.concatenate([np.asarray(m[name]) for m in in_maps], axis=0)
        for name in in_names
    ] + [np.zeros((n_cores * z.shape[0], *z.shape[1:]), z.dtype) for z in zero_outs]
    dev_in = [jax.device_put(a, sh) for a in concat]
    return fn, dev_in, out_names, out_avals


def kernel(
    x_concepts_encoded, relevance_weights, bn_gamma, bn_beta, bn_mean, bn_var, W, b
):
    global _NC_CACHE, LAST_RESULT
    x_np = np.asarray(x_concepts_encoded, dtype=np.float32)
    w_np = np.asarray(relevance_weights, dtype=np.float32)
    if _NC_CACHE is None:
        _NC_CACHE = build_kernel()
    nc = _NC_CACHE
    in_maps = make_host_inputs(
        x_np,
        w_np,
        np.asarray(bn_gamma, dtype=np.float32),
        np.asarray(bn_beta, dtype=np.float32),
        np.asarray(bn_mean, dtype=np.float32),
        np.asarray(bn_var, dtype=np.float32),
        np.asarray(W, dtype=np.float32),
        np.asarray(b, dtype=np.float32),
    )
    trace = bool(int(os.environ.get("KERNEL_TRACE", "0")))
    LAST_RESULT = run_bass_kernel_spmd(
        nc, in_maps, core_ids=list(range(N_CORES)), trace=trace
    )
    outs = []
    for i in range(N_CORES):
        # out_d[sb, nn, p, half*128+bb] -> out[b_loc, do]
        o = np.asarray(LAST_RESULT.results[i]["out"]).astype(np.float32)
        o = o.reshape(NSB, NN, 128, 2, 128).transpose(0, 1, 4, 3, 2)
        outs.append(o.reshape(B_LOC, DO))
    return np.concatenate(outs, axis=0)


# revision 13
# speedup vs baseline: 1.4658x; 1.4658x over previous
"""Trainium2 Bass kernel for nn_Aggregator (BN1d + Swish + Linear + relevance-weighted head sum).

out[b, :] = sum_h w[b,h] * (silu(x[b,h,:] * inv + shift) @ W.T + bias)
          = (sum_h w[b,h] * silu(x[b,h,:] * inv + shift)) @ W.T + (sum_h w[b,h]) * bias

Data parallel over 8 NeuronCores: batch dim B split 8 ways, all params replicated.

v2 design — bf16 streaming + feature-transposed layout:
  - x is cast to bf16 host-side (tolerance 2e-2 >> bf16 error ~5e-3): halves
    the mandatory HBM read from 16.8 MB to 8.4 MB per core.
  - Features on PARTITIONS (d = c*128 + p), rows on the free dim. Per-core
    free layout per superblock (sb = 256 b-values): [c(4), nn(2), h(8), b(128)].
  - BN affine = one dual-scalar DVE/GPSIMD tensor_scalar per c-chunk
    (scale=inv_p, then add shift_p), 2x bf16 rate on DVE.
  - Swish = ONE big ACT instruction per superblock (N=8192/partition) —
    amortizes the 352-cycle ACT instruction overhead. ACT is the bottleneck
    engine at ~28.5 us/rep.
  - Weighted head-sum: DVE mult by w (broadcast across partitions) + 3-level
    contiguous tree add over h. No PE staircase, no transposes.
  - Linear: 16 small matmuls/sb (stationary W chunk [128d,128do], moving
    g [128d,128b]) accumulating over c in PSUM. PE ~5 us total.
  - bias: out = psum + b_do * sumw_b via one scalar_tensor_tensor per half.
  - Output stored bf16, transposed [do, b]; host unscrambles + upcasts.
"""

import os
from contextlib import ExitStack

import numpy as np

import concourse.bacc as bacc
import concourse.mybir as mybir
import concourse.tile as tile
from concourse.bass_utils import run_bass_kernel_spmd
from concourse.mybir import AluOpType

N_CORES = 8
B, H, D, DO = 8192, 8, 512, 256
B_LOC = B // N_CORES            # 1024 b-values per core
NSB = 4                         # superblocks of 256 b-values
SB_B = B_LOC // NSB             # 256 b per superblock
NN = 2                          # blocks of 128 b per superblock
NC_CHUNK = 4                    # feature chunks of 128
FREE = NC_CHUNK * NN * H * 128  # 8192 elems per partition per superblock
EPS = 1e-5
FP = mybir.dt.float32
BF = mybir.dt.bfloat16

# Which BN c-chunks run on GPSIMD (rest on DVE). Balancing knob.
GP_BN_CS = (2, 3)


ALL_STAGES = frozenset({"bn", "silu", "wmul", "tree", "mm", "tail"})


def build_kernel(repeat: int = 1, stages: frozenset = ALL_STAGES):
    """repeat>1 re-runs the whole superblock loop (same I/O) for slope timing.

    stages: ablation knob for timing attribution; dropping stages yields a
    garbage-output (but schedulable) kernel.
    """
    nc = bacc.Bacc(
        "TRN2",
        target_bir_lowering=False,
        debug=False,
        num_devices=N_CORES,
    )

    x_d = nc.dram_tensor("x", (NSB, 128, FREE), BF, kind="ExternalInput")
    wb_d = nc.dram_tensor("wb", (128, NSB * NN * H * 128), BF, kind="ExternalInput")
    sumw_d = nc.dram_tensor("sumw", (128, NSB * NN * 128), FP, kind="ExternalInput")
    invT_d = nc.dram_tensor("invT", (128, NC_CHUNK), FP, kind="ExternalInput")
    shiftT_d = nc.dram_tensor("shiftT", (128, NC_CHUNK), FP, kind="ExternalInput")
    wt_d = nc.dram_tensor("wt", (128, 2 * NC_CHUNK * 128), BF, kind="ExternalInput")
    bvec_d = nc.dram_tensor("bvec", (128, 2), FP, kind="ExternalInput")
    out_d = nc.dram_tensor("out", (NSB, NN, 128, DO), BF, kind="ExternalOutput")

    with tile.TileContext(nc) as tc, ExitStack() as ctx:
        const = ctx.enter_context(tc.tile_pool(name="const", bufs=1))
        xpool = ctx.enter_context(tc.tile_pool(name="xin", bufs=3))

        # first superblock load precedes const loads in the SP FIFO except the
        # tiny BN params the first compute depends on
        invT = const.tile([128, NC_CHUNK], FP)
        nc.sync.dma_start(invT[:], invT_d.ap())
        shiftT = const.tile([128, NC_CHUNK], FP)
        nc.sync.dma_start(shiftT[:], shiftT_d.ap())
        xt0 = xpool.tile([128, FREE], BF, tag="xt")
        nc.sync.dma_start(xt0[:], x_d.ap()[0])
        wbc = const.tile([128, NSB * NN * H * 128], BF)
        nc.sync.dma_start(wbc[:], wb_d.ap())
        wt = const.tile([128, 2 * NC_CHUNK * 128], BF)
        nc.sync.dma_start(wt[:], wt_d.ap())
        sumw = const.tile([128, NSB * NN * 128], FP)
        nc.sync.dma_start(sumw[:], sumw_d.ap())
        bvec = const.tile([128, 2], FP)
        nc.sync.dma_start(bvec[:], bvec_d.ap())

        tpool = ctx.enter_context(tc.tile_pool(name="bn", bufs=2))
        hspool = ctx.enter_context(tc.tile_pool(name="hs", bufs=2))
        mpool = ctx.enter_context(tc.tile_pool(name="m", bufs=2))
        t4pool = ctx.enter_context(tc.tile_pool(name="t4", bufs=2))
        t2pool = ctx.enter_context(tc.tile_pool(name="t2", bufs=2))
        gpool = ctx.enter_context(tc.tile_pool(name="g", bufs=2))
        opool = ctx.enter_context(tc.tile_pool(name="o", bufs=4))
        pspool = ctx.enter_context(tc.tile_pool(name="ps", bufs=2, space="PSUM"))

        CH = FREE // NC_CHUNK  # 2048 elems per c-chunk per partition

        # --- software-pipelined schedule -------------------------------------
        # Engines execute their instruction streams IN ORDER, so naive
        # stage-after-stage issue makes DVE stall at wmul(k) waiting for ACT's
        # silu(k), while ACT then waits for DVE's bn(k+1): a serial ping-pong.
        # Skewing the issue by one step per pipeline stage keeps every
        # engine's next instruction dependent only on work issued >=1 step
        # earlier, which is already done in steady state.
        #   step i: dma(i+1) | bn(i) silu(i) | stt(i-2)+store(i-2) | wmul/tree/mm(i-1)

        live = {}

        def load(it):
            if it == 0:
                xt = xt0
            else:
                xt = xpool.tile([128, FREE], BF, tag="xt", name="xt")
                nc.sync.dma_start(xt[:], x_d.ap()[it % NSB])
            live[it] = {"xt": xt}

        def head(it):
            sb = it % NSB
            xt = live[it]["xt"]
            t = tpool.tile([128, FREE], BF, tag="t", name="t")
            if "bn" in stages:
                for c in range(NC_CHUNK):
                    eng = nc.gpsimd if c in GP_BN_CS else nc.vector
                    eng.tensor_scalar(
                        t[:, c * CH : (c + 1) * CH],
                        xt[:, c * CH : (c + 1) * CH],
                        invT[:, c : c + 1],
                        shiftT[:, c : c + 1],
                        AluOpType.mult,
                        AluOpType.add,
                    )
            hs = hspool.tile([128, FREE], BF, tag="hs", name="hs")
            if "silu" in stages:
                src = t if "bn" in stages else xt
                nc.scalar.activation(
                    hs[:], src[:], mybir.ActivationFunctionType.Silu
                )
            live[it]["hs"] = hs

        def mid(it):
            sb = it % NSB
            st = live[it]
            # weighted by relevance: m = hs * w (w broadcast over partitions+c)
            m = mpool.tile([128, FREE], BF, tag="m", name="m")
            if "wmul" in stages:
                src = st["hs"] if "silu" in stages else st["xt"]
                wslice = (
                    wbc[:, sb * CH : (sb + 1) * CH]
                    .unsqueeze(1)
                    .broadcast_to([128, NC_CHUNK, CH])
                )
                nc.vector.tensor_tensor(
                    m[:].rearrange("p (c r) -> p c r", c=NC_CHUNK),
                    src[:].rearrange("p (c r) -> p c r", c=NC_CHUNK),
                    wslice,
                    AluOpType.mult,
                )
            # head-sum: 3-level tree over h (cn = 2c+nn merged dim of 8)
            g = None
            if "tree" in stages:
                g = gpool.tile([128, 8 * 128], BF, tag="g", name="g")
                src = m if "wmul" in stages else st["xt"]
                mv = src[:].rearrange("p (cn h b) -> p cn h b", cn=8, h=8)
                t4 = t4pool.tile([128, 8 * 4 * 128], BF, tag="t4", name="t4")
                t4v = t4[:].rearrange("p (cn h b) -> p cn h b", cn=8, h=4)
                nc.vector.tensor_tensor(
                    t4v, mv[:, :, 0:4, :], mv[:, :, 4:8, :], AluOpType.add
                )
                t2t = t2pool.tile([128, 8 * 2 * 128], BF, tag="t2", name="t2")
                t2v = t2t[:].rearrange("p (cn h b) -> p cn h b", cn=8, h=2)
                nc.vector.tensor_tensor(
                    t2v, t4v[:, :, 0:2, :], t4v[:, :, 2:4, :], AluOpType.add
                )
                gv = g[:].rearrange("p (cn h b) -> p cn h b", cn=8, h=1)
                nc.vector.tensor_tensor(
                    gv, t2v[:, :, 0:1, :], t2v[:, :, 1:2, :], AluOpType.add
                )
            # Linear: accumulate over c into PSUM per (nn, half)
            ps = [
                pspool.tile([128, 128], FP, tag=f"ps{i}", name=f"ps{i}")
                for i in range(NN * 2)
            ]
            if "mm" in stages:
                for c in range(NC_CHUNK):
                    for half in range(2):
                        wsl = wt[:, (2 * c + half) * 128 : (2 * c + half + 1) * 128]
                        for n2 in range(NN):
                            cn = 2 * c + n2
                            rhs = (
                                g[:, cn * 128 : (cn + 1) * 128]
                                if "tree" in stages
                                else wt[:, cn * 128 : (cn + 1) * 128]
                            )
                            nc.tensor.matmul(
                                ps[n2 * 2 + half][:],
                                wsl,
                                rhs,
                                start=(c == 0),
                                stop=(c == NC_CHUNK - 1),
                            )
            st["ps"] = ps

        def tail(it):
            sb = it % NSB
            st = live.pop(it)
            if "tail" not in stages:
                return
            # bias: out = psum + b_do * sumw_b ; store bf16 via ACT HWDGE queue
            for n2 in range(NN):
                o = opool.tile([128, DO], BF, tag="o", name="o")
                for half in range(2):
                    if "mm" in stages:
                        nc.vector.scalar_tensor_tensor(
                            o[:, half * 128 : (half + 1) * 128],
                            sumw[:, (sb * NN + n2) * 128 : (sb * NN + n2 + 1) * 128],
                            bvec[:, half : half + 1],
                            st["ps"][n2 * 2 + half][:],
                            AluOpType.mult,
                            AluOpType.add,
                        )
                    else:
                        nc.vector.tensor_scalar(
                            o[:, half * 128 : (half + 1) * 128],
                            wt[:, half * 128 : (half + 1) * 128],
                            bvec[:, half : half + 1],
                            None,
                            AluOpType.mult,
                        )
                nc.scalar.dma_start(out_d.ap()[sb][n2], o[:])

        T = repeat * NSB
        load(0)
        for i in range(T + 2):
            if i + 1 < T:
                load(i + 1)
            if i < T:
                head(i)
            if i >= 2:
                tail(i - 2)
            if 1 <= i <= T:
                mid(i - 1)

    nc.compile()
    return nc


def make_host_inputs(x_np, w_np, gamma, beta, mean, var, W, b):
    """Build the per-core input maps (host-side layout prep only)."""
    import ml_dtypes

    BFH = ml_dtypes.bfloat16
    inv = (gamma / np.sqrt(var + EPS)).astype(np.float32)
    shift = (beta - mean * inv).astype(np.float32)
    invT = np.ascontiguousarray(inv.reshape(NC_CHUNK, 128).T)
    shiftT = np.ascontiguousarray(shift.reshape(NC_CHUNK, 128).T)
    # Wt[p, (c*2+half)*128 + q] = W[half*128+q, c*128+p]
    wt = np.ascontiguousarray(
        W.astype(np.float32)
        .reshape(2, 128, NC_CHUNK, 128)  # [half, q, c, p]
        .transpose(3, 2, 0, 1)           # [p, c, half, q]
        .reshape(128, 2 * NC_CHUNK * 128)
        .astype(BFH)
    )
    bvec = np.ascontiguousarray(b.astype(np.float32).reshape(2, 128).T)

    xb = x_np.astype(BFH)  # round once, globally
    in_maps = []
    for core in range(N_CORES):
        b0 = core * B_LOC
        # x_t[sb, p, c, nn, h, bb] = x[b0 + sb*256 + nn*128 + bb, h, c*128 + p]
        x_core = xb[b0 : b0 + B_LOC].reshape(NSB, NN, 128, H, NC_CHUNK, 128)
        x_t = np.ascontiguousarray(
            x_core.transpose(0, 5, 4, 1, 3, 2).reshape(NSB, 128, FREE)
        )
        w_core = w_np[b0 : b0 + B_LOC].astype(np.float32)
        # wb[p, sb, nn, h, bb]
        w_r = (
            w_core.reshape(NSB, NN, 128, H).transpose(0, 1, 3, 2).reshape(-1)
        )
        wb = np.ascontiguousarray(
            np.broadcast_to(w_r.astype(BFH), (128, NSB * NN * H * 128))
        )
        sumw_r = w_core.sum(axis=1).astype(np.float32)  # [1024] = [sb, nn, bb]
        sumw = np.ascontiguousarray(np.broadcast_to(sumw_r, (128, NSB * NN * 128)))
        in_maps.append(
            {
                "x": x_t,
                "wb": wb,
                "sumw": sumw,
                "invT": invT,
                "shiftT": shiftT,
                "wt": wt,
                "bvec": bvec,
            }
        )
    return in_maps


_NC_CACHE = None
LAST_RESULT = None


def make_runner(nc, in_maps):
    """Build a reusable jitted SPMD callable with device-resident inputs.

    Mirrors bass2jax.run_bass_via_pjrt's multi-core path, but without
    donation so the same device buffers can be executed repeatedly for
    steady-state timing.
    """
    import jax
    from concourse import bass2jax
    from jax.experimental.shard_map import shard_map
    from jax.sharding import Mesh, NamedSharding, PartitionSpec

    bass2jax.install_neuronx_cc_hook()
    partition_name = nc.partition_id_tensor.name if nc.partition_id_tensor else None
    in_names, out_names, out_avals, zero_outs = [], [], [], []
    for alloc in nc.m.functions[0].allocations:
        if not isinstance(alloc, mybir.MemoryLocationSet):
            continue
        name = alloc.memorylocations[0].name
        if alloc.kind == "ExternalInput":
            if name != partition_name:
                in_names.append(name)
        elif alloc.kind == "ExternalOutput":
            out_names.append(name)
            shape = tuple(alloc.tensor_shape)
            dtype = mybir.dt.np(alloc.dtype)
            out_avals.append(jax.core.ShapedArray(shape, dtype))
            zero_outs.append(np.zeros(shape, dtype))
    n_params = len(in_names)
    all_names = in_names + out_names
    if partition_name is not None:
        all_names = all_names + [partition_name]

    def _body(*args):
        operands = list(args)
        if partition_name is not None:
            operands.append(bass2jax.partition_id_tensor())
        outs = bass2jax._bass_exec_p.bind(
            *operands,
            out_avals=tuple(out_avals),
            in_names=tuple(all_names),
            out_names=tuple(out_names),
            lowering_input_output_aliases=(),
            sim_require_finite=True,
            sim_require_nnan=True,
            nc=nc,
        )
        return tuple(outs)

    n_cores = len(in_maps)
    devices = jax.devices()[:n_cores]
    mesh = Mesh(np.asarray(devices), ("core",))
    in_specs = (PartitionSpec("core"),) * (n_params + len(out_names))
    out_specs = (PartitionSpec("core"),) * len(out_names)
    fn = jax.jit(
        shard_map(_body, mesh=mesh, in_specs=in_specs, out_specs=out_specs,
                  check_rep=False),
        keep_unused=True,
    )
    sh = NamedSharding(mesh, PartitionSpec("core"))
    concat = [
        np.concatenate([np.asarray(m[name]) for m in in_maps], axis=0)
        for name in in_names
    ] + [np.zeros((n_cores * z.shape[0], *z.shape[1:]), z.dtype) for z in zero_outs]
    dev_in = [jax.device_put(a, sh) for a in concat]
    return fn, dev_in, out_names, out_avals


def kernel(
    x_concepts_encoded, relevance_weights, bn_gamma, bn_beta, bn_mean, bn_var, W, b
):
    global _NC_CACHE, LAST_RESULT
    x_np = np.asarray(x_concepts_encoded, dtype=np.float32)
    w_np = np.asarray(relevance_weights, dtype=np.float32)
    if _NC_CACHE is None:
        _NC_CACHE = build_kernel()
    nc = _NC_CACHE
    in_maps = make_host_inputs(
        x_np,
        w_np,
        np.asarray(bn_gamma, dtype=np.float32),
        np.asarray(bn_beta, dtype=np.float32),
        np.asarray(bn_mean, dtype=np.float32),
        np.asarray(bn_var, dtype=np.float32),
        np.asarray(W, dtype=np.float32),
        np.asarray(b, dtype=np.float32),
    )
    trace = bool(int(os.environ.get("KERNEL_TRACE", "0")))
    LAST_RESULT = run_bass_kernel_spmd(
        nc, in_maps, core_ids=list(range(N_CORES)), trace=trace
    )
    outs = []
    for i in range(N_CORES):
        # out_d[sb, nn, p, half*128+bb] -> out[b_loc, do]
        o = np.asarray(LAST_RESULT.results[i]["out"]).astype(np.float32)
        o = o.reshape(NSB, NN, 128, 2, 128).transpose(0, 1, 4, 3, 2)
        outs.append(o.reshape(B_LOC, DO))
    return np.concatenate(outs, axis=0)


# revision 17
# speedup vs baseline: 1.8021x; 1.2294x over previous
"""Trainium2 Bass kernel for nn_Aggregator (BN1d + Swish + Linear + relevance-weighted head sum).

out[b, :] = sum_h w[b,h] * (silu(x[b,h,:] * inv + shift) @ W.T + bias)
          = (sum_h w[b,h] * silu(x[b,h,:] * inv + shift)) @ W.T + (sum_h w[b,h]) * bias

Data parallel over 8 NeuronCores: batch dim B split 8 ways, all params replicated.

v2 design — bf16 streaming + feature-transposed layout:
  - x is cast to bf16 host-side (tolerance 2e-2 >> bf16 error ~5e-3): halves
    the mandatory HBM read from 16.8 MB to 8.4 MB per core.
  - Features on PARTITIONS (d = c*128 + p), rows on the free dim. Per-core
    free layout per superblock (sb = 256 b-values): [c(4), nn(2), h(8), b(128)].
  - BN affine = one dual-scalar DVE/GPSIMD tensor_scalar per c-chunk
    (scale=inv_p, then add shift_p), 2x bf16 rate on DVE.
  - Swish = ONE big ACT instruction per superblock (N=8192/partition) —
    amortizes the 352-cycle ACT instruction overhead. ACT is the bottleneck
    engine at ~28.5 us/rep.
  - Weighted head-sum: DVE mult by w (broadcast across partitions) + 3-level
    contiguous tree add over h. No PE staircase, no transposes.
  - Linear: 16 small matmuls/sb (stationary W chunk [128d,128do], moving
    g [128d,128b]) accumulating over c in PSUM. PE ~5 us total.
  - bias: out = psum + b_do * sumw_b via one scalar_tensor_tensor per half.
  - Output stored bf16, transposed [do, b]; host unscrambles + upcasts.
"""

import os
from contextlib import ExitStack

import numpy as np

import concourse.bacc as bacc
import concourse.mybir as mybir
import concourse.tile as tile
from concourse.bass_utils import run_bass_kernel_spmd
from concourse.mybir import AluOpType

N_CORES = 8
B, H, D, DO = 8192, 8, 512, 256
B_LOC = B // N_CORES            # 1024 b-values per core
NSB = 4                         # superblocks of 256 b-values
SB_B = B_LOC // NSB             # 256 b per superblock
NN = 2                          # blocks of 128 b per superblock
NC_CHUNK = 4                    # feature chunks of 128
FREE = NC_CHUNK * NN * H * 128  # 8192 elems per partition per superblock
EPS = 1e-5
FP = mybir.dt.float32
BF = mybir.dt.bfloat16

# Which BN c-chunks run on GPSIMD (rest on DVE). Balancing knob.
GP_BN_CS = (1, 2)


ALL_STAGES = frozenset({"bn", "silu", "wmul", "tree", "mm", "tail"})


def build_kernel(repeat: int = 1, stages: frozenset = ALL_STAGES):
    """repeat>1 re-runs the whole superblock loop (same I/O) for slope timing.

    stages: ablation knob for timing attribution; dropping stages yields a
    garbage-output (but schedulable) kernel.
    """
    nc = bacc.Bacc(
        "TRN2",
        target_bir_lowering=False,
        debug=False,
        num_devices=N_CORES,
    )

    x_d = nc.dram_tensor("x", (NSB, 128, FREE), BF, kind="ExternalInput")
    wb_d = nc.dram_tensor("wb", (128, NSB * NN * H * 128), BF, kind="ExternalInput")
    sumw_d = nc.dram_tensor("sumw", (128, NSB * NN * 128), FP, kind="ExternalInput")
    invT_d = nc.dram_tensor("invT", (128, NC_CHUNK), FP, kind="ExternalInput")
    shiftT_d = nc.dram_tensor("shiftT", (128, NC_CHUNK), FP, kind="ExternalInput")
    wt_d = nc.dram_tensor("wt", (128, 2 * NC_CHUNK * 128), BF, kind="ExternalInput")
    bvec_d = nc.dram_tensor("bvec", (128, 2), FP, kind="ExternalInput")
    out_d = nc.dram_tensor("out", (NSB, NN, 128, DO), BF, kind="ExternalOutput")

    with tile.TileContext(nc) as tc, ExitStack() as ctx:
        const = ctx.enter_context(tc.tile_pool(name="const", bufs=1))
        xpool = ctx.enter_context(tc.tile_pool(name="xin", bufs=3))

        # first superblock load precedes const loads in the SP FIFO except the
        # tiny BN params the first compute depends on
        invT = const.tile([128, NC_CHUNK], FP)
        nc.sync.dma_start(invT[:], invT_d.ap())
        shiftT = const.tile([128, NC_CHUNK], FP)
        nc.sync.dma_start(shiftT[:], shiftT_d.ap())
        xt0 = xpool.tile([128, FREE], BF, tag="xt")
        nc.sync.dma_start(xt0[:], x_d.ap()[0])
        wbc = const.tile([128, NSB * NN * H * 128], BF)
        nc.sync.dma_start(wbc[:], wb_d.ap())
        wt = const.tile([128, 2 * NC_CHUNK * 128], BF)
        nc.sync.dma_start(wt[:], wt_d.ap())
        sumw = const.tile([128, NSB * NN * 128], FP)
        nc.sync.dma_start(sumw[:], sumw_d.ap())
        bvec = const.tile([128, 2], FP)
        nc.sync.dma_start(bvec[:], bvec_d.ap())

        tpool = ctx.enter_context(tc.tile_pool(name="bn", bufs=2))
        hspool = ctx.enter_context(tc.tile_pool(name="hs", bufs=2))
        mpool = ctx.enter_context(tc.tile_pool(name="m", bufs=2))
        t4pool = ctx.enter_context(tc.tile_pool(name="t4", bufs=2))
        t2pool = ctx.enter_context(tc.tile_pool(name="t2", bufs=2))
        gpool = ctx.enter_context(tc.tile_pool(name="g", bufs=2))
        opool = ctx.enter_context(tc.tile_pool(name="o", bufs=4))
        pspool = ctx.enter_context(tc.tile_pool(name="ps", bufs=2, space="PSUM"))

        CH = FREE // NC_CHUNK  # 2048 elems per c-chunk per partition

        # --- software-pipelined schedule -------------------------------------
        # Engines execute their instruction streams IN ORDER, so naive
        # stage-after-stage issue makes DVE stall at wmul(k) waiting for ACT's
        # silu(k), while ACT then waits for DVE's bn(k+1): a serial ping-pong.
        # Skewing the issue by one step per pipeline stage keeps every
        # engine's next instruction dependent only on work issued >=1 step
        # earlier, which is already done in steady state.
        #   step i: dma(i+1) | bn(i) silu(i) | stt(i-2)+store(i-2) | wmul/tree/mm(i-1)

        live = {}

        def load(it):
            if it == 0:
                xt = xt0
            else:
                xt = xpool.tile([128, FREE], BF, tag="xt", name="xt")
                nc.sync.dma_start(xt[:], x_d.ap()[it % NSB])
            live[it] = {"xt": xt}

        def head(it):
            sb = it % NSB
            xt = live[it]["xt"]
            t = tpool.tile([128, FREE], BF, tag="t", name="t")
            if "bn" in stages:
                for c in range(NC_CHUNK):
                    eng = nc.gpsimd if c in GP_BN_CS else nc.vector
                    eng.tensor_scalar(
                        t[:, c * CH : (c + 1) * CH],
                        xt[:, c * CH : (c + 1) * CH],
                        invT[:, c : c + 1],
                        shiftT[:, c : c + 1],
                        AluOpType.mult,
                        AluOpType.add,
                    )
            hs = hspool.tile([128, FREE], BF, tag="hs", name="hs")
            if "silu" in stages:
                src = t if "bn" in stages else xt
                nc.scalar.activation(
                    hs[:], src[:], mybir.ActivationFunctionType.Silu
                )
            live[it]["hs"] = hs

        def mid(it):
            sb = it % NSB
            st = live[it]
            # weighted by relevance: m = hs * w (w broadcast over partitions+c)
            m = mpool.tile([128, FREE], BF, tag="m", name="m")
            if "wmul" in stages:
                src = st["hs"] if "silu" in stages else st["xt"]
                wslice = (
                    wbc[:, sb * CH : (sb + 1) * CH]
                    .unsqueeze(1)
                    .broadcast_to([128, NC_CHUNK, CH])
                )
                nc.vector.tensor_tensor(
                    m[:].rearrange("p (c r) -> p c r", c=NC_CHUNK),
                    src[:].rearrange("p (c r) -> p c r", c=NC_CHUNK),
                    wslice,
                    AluOpType.mult,
                )
            # head-sum level 1 on DVE (h 8->4); levels 2+3 ride the PE's PSUM
            # accumulation in the Linear below (4 extra rhs columns per ps)
            t4 = None
            if "tree" in stages:
                src = m if "wmul" in stages else st["xt"]
                mv = src[:].rearrange("p (cn h b) -> p cn h b", cn=8, h=8)
                t4 = t4pool.tile([128, 8 * 4 * 128], BF, tag="t4", name="t4")
                t4v = t4[:].rearrange("p (cn h b) -> p cn h b", cn=8, h=4)
                nc.vector.tensor_tensor(
                    t4v, mv[:, :, 0:4, :], mv[:, :, 4:8, :], AluOpType.add
                )
            # Linear: accumulate over c AND remaining 4 h-pairs into PSUM
            ps = [
                pspool.tile([128, 128], FP, tag=f"ps{i}", name=f"ps{i}")
                for i in range(NN * 2)
            ]
            if "mm" in stages:
                for c in range(NC_CHUNK):
                    for half in range(2):
                        wsl = wt[:, (2 * c + half) * 128 : (2 * c + half + 1) * 128]
                        for n2 in range(NN):
                            cn = 2 * c + n2
                            for hh in range(4):
                                rhs = (
                                    t4[:, (cn * 4 + hh) * 128 : (cn * 4 + hh + 1) * 128]
                                    if "tree" in stages
                                    else wt[:, (2 * c + half) * 128 : (2 * c + half + 1) * 128]
                                )
                                nc.tensor.matmul(
                                    ps[n2 * 2 + half][:],
                                    wsl,
                                    rhs,
                                    start=(c == 0 and hh == 0),
                                    stop=(c == NC_CHUNK - 1 and hh == 3),
                                )
            st["ps"] = ps

        def tail(it):
            sb = it % NSB
            st = live.pop(it)
            if "tail" not in stages:
                return
            # bias: out = psum + b_do * sumw_b ; store bf16 via ACT HWDGE queue
            for n2 in range(NN):
                o = opool.tile([128, DO], BF, tag="o", name="o")
                for half in range(2):
                    if "mm" in stages:
                        nc.vector.scalar_tensor_tensor(
                            o[:, half * 128 : (half + 1) * 128],
                            sumw[:, (sb * NN + n2) * 128 : (sb * NN + n2 + 1) * 128],
                            bvec[:, half : half + 1],
                            st["ps"][n2 * 2 + half][:],
                            AluOpType.mult,
                            AluOpType.add,
                        )
                    else:
                        nc.vector.tensor_scalar(
                            o[:, half * 128 : (half + 1) * 128],
                            wt[:, half * 128 : (half + 1) * 128],
                            bvec[:, half : half + 1],
                            None,
                            AluOpType.mult,
                        )
                nc.scalar.dma_start(out_d.ap()[sb][n2], o[:])

        T = repeat * NSB
        load(0)
        for i in range(T + 2):
            if i + 1 < T:
                load(i + 1)
            if i < T:
                head(i)
            if i >= 2:
                tail(i - 2)
            if 1 <= i <= T:
                mid(i - 1)

    nc.compile()
    return nc


def make_host_inputs(x_np, w_np, gamma, beta, mean, var, W, b):
    """Build the per-core input maps (host-side layout prep only)."""
    import ml_dtypes

    BFH = ml_dtypes.bfloat16
    inv = (gamma / np.sqrt(var + EPS)).astype(np.float32)
    shift = (beta - mean * inv).astype(np.float32)
    invT = np.ascontiguousarray(inv.reshape(NC_CHUNK, 128).T)
    shiftT = np.ascontiguousarray(shift.reshape(NC_CHUNK, 128).T)
    # Wt[p, (c*2+half)*128 + q] = W[half*128+q, c*128+p]
    wt = np.ascontiguousarray(
        W.astype(np.float32)
        .reshape(2, 128, NC_CHUNK, 128)  # [half, q, c, p]
        .transpose(3, 2, 0, 1)           # [p, c, half, q]
        .reshape(128, 2 * NC_CHUNK * 128)
        .astype(BFH)
    )
    bvec = np.ascontiguousarray(b.astype(np.float32).reshape(2, 128).T)

    xb = x_np.astype(BFH)  # round once, globally
    in_maps = []
    for core in range(N_CORES):
        b0 = core * B_LOC
        # x_t[sb, p, c, nn, h, bb] = x[b0 + sb*256 + nn*128 + bb, h, c*128 + p]
        x_core = xb[b0 : b0 + B_LOC].reshape(NSB, NN, 128, H, NC_CHUNK, 128)
        x_t = np.ascontiguousarray(
            x_core.transpose(0, 5, 4, 1, 3, 2).reshape(NSB, 128, FREE)
        )
        w_core = w_np[b0 : b0 + B_LOC].astype(np.float32)
        # wb[p, sb, nn, h, bb]
        w_r = (
            w_core.reshape(NSB, NN, 128, H).transpose(0, 1, 3, 2).reshape(-1)
        )
        wb = np.ascontiguousarray(
            np.broadcast_to(w_r.astype(BFH), (128, NSB * NN * H * 128))
        )
        sumw_r = w_core.sum(axis=1).astype(np.float32)  # [1024] = [sb, nn, bb]
        sumw = np.ascontiguousarray(np.broadcast_to(sumw_r, (128, NSB * NN * 128)))
        in_maps.append(
            {
                "x": x_t,
                "wb": wb,
                "sumw": sumw,
                "invT": invT,
                "shiftT": shiftT,
                "wt": wt,
                "bvec": bvec,
            }
        )
    return in_maps


_NC_CACHE = None
LAST_RESULT = None


def make_runner(nc, in_maps):
    """Build a reusable jitted SPMD callable with device-resident inputs.

    Mirrors bass2jax.run_bass_via_pjrt's multi-core path, but without
    donation so the same device buffers can be executed repeatedly for
    steady-state timing.
    """
    import jax
    from concourse import bass2jax
    from jax.experimental.shard_map import shard_map
    from jax.sharding import Mesh, NamedSharding, PartitionSpec

    bass2jax.install_neuronx_cc_hook()
    partition_name = nc.partition_id_tensor.name if nc.partition_id_tensor else None
    in_names, out_names, out_avals, zero_outs = [], [], [], []
    for alloc in nc.m.functions[0].allocations:
        if not isinstance(alloc, mybir.MemoryLocationSet):
            continue
        name = alloc.memorylocations[0].name
        if alloc.kind == "ExternalInput":
            if name != partition_name:
                in_names.append(name)
        elif alloc.kind == "ExternalOutput":
            out_names.append(name)
            shape = tuple(alloc.tensor_shape)
            dtype = mybir.dt.np(alloc.dtype)
            out_avals.append(jax.core.ShapedArray(shape, dtype))
            zero_outs.append(np.zeros(shape, dtype))
    n_params = len(in_names)
    all_names = in_names + out_names
    if partition_name is not None:
        all_names = all_names + [partition_name]

    def _body(*args):
        operands = list(args)
        if partition_name is not None:
            operands.append(bass2jax.partition_id_tensor())
        outs = bass2jax._bass_exec_p.bind(
            *operands,
            out_avals=tuple(out_avals),
            in_names=tuple(all_names),
            out_names=tuple(out_names),
            lowering_input_output_aliases=(),
            sim_require_finite=True,
            sim_require_nnan=True,
            nc=nc,
        )
        return tuple(outs)

    n_cores = len(in_maps)
    devices = jax.devices()[:n_cores]
    mesh = Mesh(np.asarray(devices), ("core",))
    in_specs = (PartitionSpec("core"),) * (n_params + len(out_names))
    out_specs = (PartitionSpec("core"),) * len(out_names)
    fn = jax.jit(
        shard_map(_body, mesh=mesh, in_specs=in_specs, out_specs=out_specs,
                  check_rep=False),
        keep_unused=True,
    )
    sh = NamedSharding(mesh, PartitionSpec("core"))
    concat = [
        np.concatenate([np.asarray(m[name]) for m in in_maps], axis=0)
        for name in in_names
    ] + [np.zeros((n_cores * z.shape[0], *z.shape[1:]), z.dtype) for z in zero_outs]
    dev_in = [jax.device_put(a, sh) for a in concat]
    return fn, dev_in, out_names, out_avals


def kernel(
    x_concepts_encoded, relevance_weights, bn_gamma, bn_beta, bn_mean, bn_var, W, b
):
    global _NC_CACHE, LAST_RESULT
    x_np = np.asarray(x_concepts_encoded, dtype=np.float32)
    w_np = np.asarray(relevance_weights, dtype=np.float32)
    if _NC_CACHE is None:
        _NC_CACHE = build_kernel()
    nc = _NC_CACHE
    in_maps = make_host_inputs(
        x_np,
        w_np,
        np.asarray(bn_gamma, dtype=np.float32),
        np.asarray(bn_beta, dtype=np.float32),
        np.asarray(bn_mean, dtype=np.float32),
        np.asarray(bn_var, dtype=np.float32),
        np.asarray(W, dtype=np.float32),
        np.asarray(b, dtype=np.float32),
    )
    trace = bool(int(os.environ.get("KERNEL_TRACE", "0")))
    LAST_RESULT = run_bass_kernel_spmd(
        nc, in_maps, core_ids=list(range(N_CORES)), trace=trace
    )
    outs = []
    for i in range(N_CORES):
        # out_d[sb, nn, p, half*128+bb] -> out[b_loc, do]
        o = np.asarray(LAST_RESULT.results[i]["out"]).astype(np.float32)
        o = o.reshape(NSB, NN, 128, 2, 128).transpose(0, 1, 4, 3, 2)
        outs.append(o.reshape(B_LOC, DO))
    return np.concatenate(outs, axis=0)
